# revision 6
# baseline (speedup 1.0000x reference)
"""Trainium2 Bass kernel for nn_CAMLocalHead (CAM target + conv head + BCE).

Self-contained: takes FULL inputs, shards batch B=8 across 8 NeuronCores
(one sample per core), runs a Bass/Tile kernel per core, sums the per-core
partial BCE sums on host.
"""
import sys

for _p in ("/opt/trn_rl_repo", "/opt/pypackages"):
    if _p not in sys.path:
        sys.path.append(_p)

import numpy as np
import ml_dtypes

# Problem dims (hardcoded per spec)
B, C, T, H, W = 8, 2048, 16, 7, 7
K, D = 400, 512
N_TOKEN = 392
P = 128
CT = C // P          # 16 c-tiles
DT = D // P          # 4 d-tiles
NH = 2               # spatial halves (t 0..7, 8..15)
TH = T // NH         # 8
PLANE = 81           # 9x9 padded plane
NF = TH * H * W      # 392 positions per half
NPOS = T * H * W     # 784
PADN = 7 * P         # 896 (784 padded to 7 chunks of 128)
NEG = -1.0e30

_cache = {}


def _build_nc():
    import concourse.bacc as bacc
    import concourse.mybir as mybir
    from concourse import tile

    f32 = mybir.dt.float32
    bf16 = mybir.dt.bfloat16
    AX = mybir.AxisListType.X
    OP = mybir.AluOpType
    AF = mybir.ActivationFunctionType

    nc = bacc.Bacc(trn_type="TRN2")

    xpad_d = nc.dram_tensor("xpad", [CT, P, T * PLANE], bf16, kind="ExternalInput")
    wt_d = nc.dram_tensor("wt", [DT, P, CT * 9 * P], bf16, kind="ExternalInput")
    proj_d = nc.dram_tensor("proj", [K, C], bf16, kind="ExternalInput")
    xfp_d = nc.dram_tensor("xfp", [1, K], f32, kind="ExternalInput")
    cb_d = nc.dram_tensor("cb", [P, DT], f32, kind="ExternalInput")
    sw_d = nc.dram_tensor("sw", [P, DT], bf16, kind="ExternalInput")
    sb_d = nc.dram_tensor("sb", [1, 1], f32, kind="ExternalInput")
    out_d = nc.dram_tensor("out", [1, 1], f32, kind="ExternalOutput")

    with tile.TileContext(nc) as tc:
        with (
            tc.tile_pool(name="const", bufs=1) as cp,
            tc.tile_pool(name="wp", bufs=2) as wp,
            tc.tile_pool(name="rp", bufs=4) as rp,
            tc.tile_pool(name="cps", bufs=2, space="PSUM") as cps,
            tc.tile_pool(name="sps", bufs=1, space="PSUM") as sps,
            tc.tile_pool(name="mps", bufs=2, space="PSUM") as mps,
        ):
            # ---------- constants / inputs resident in SBUF ----------
            xtiles = [cp.tile([P, T * PLANE], bf16, name=f"xt{ct}")
                      for ct in range(CT)]

            cb_sb = cp.tile([P, DT], f32)
            nc.scalar.dma_start(cb_sb[:], cb_d[:])
            sw_sb = cp.tile([P, DT], bf16)
            nc.scalar.dma_start(sw_sb[:], sw_d[:])
            sb_sb = cp.tile([1, 1], f32)
            nc.scalar.dma_start(sb_sb[:], sb_d[:])

            ones11 = cp.tile([1, 1], f32)
            nc.vector.memset(ones11[:], 1.0)
            ones_row = cp.tile([1, P], f32)
            nc.vector.memset(ones_row[:], 1.0)
            ones_col = cp.tile([P, 1], f32)
            nc.vector.memset(ones_col[:], 1.0)

            def xview(ct, tap, nh):
                dh, dw = tap // 3, tap % 3
                v = xtiles[ct][:].rearrange(
                    "p (t h w) -> p t h w", t=T, h=9, w=9)
                return v[:, nh * TH:(nh + 1) * TH, dh:dh + 7, dw:dw + 7]

            # ---------- CAM front-end (emitted between conv dt0 and dt1
            # so the DMA preamble overlaps PE work) ----------
            fe = {}

            def emit_frontend():
                proj_sb = cp.tile([P, 4 * C], bf16)
                for kc in range(4):
                    kcnt = min(P, K - kc * P)
                    nc.scalar.dma_start(
                        proj_sb[0:kcnt, kc * C:(kc + 1) * C],
                        proj_d[kc * P:kc * P + kcnt, :])
                xfp = cp.tile([1, K], f32)
                nc.scalar.dma_start(xfp[:], xfp_d[:])

                # argmax class via one-hot (sigmoid monotonic -> argmax on raw)
                m = cp.tile([1, 1], f32)
                nc.vector.reduce_max(m[:], xfp[:], axis=AX)
                oh = cp.tile([1, 4 * P], f32)
                nc.vector.memset(oh[:], 0.0)
                nc.vector.tensor_scalar(oh[0:1, 0:K], xfp[:], m[:], None,
                                        op0=OP.is_equal)
                ohT_ps = mps.tile([P, 4], f32, tag="mp")
                for i in range(4):
                    nc.tensor.transpose(ohT_ps[:, i:i + 1],
                                        oh[0:1, i * P:(i + 1) * P], ones11[:])
                ohT = cp.tile([P, 4], bf16)
                nc.vector.tensor_copy(ohT[:], ohT_ps[:])

                # w_selT[c] = proj_weight[top_cls, c], [128, CT] (c-tile cols)
                wps = mps.tile([P, CT], f32, tag="mp")
                for ct in range(CT):
                    for kc in range(4):
                        kcnt = min(P, K - kc * P)
                        nc.tensor.matmul(
                            wps[:, ct:ct + 1],
                            proj_sb[0:kcnt,
                                    kc * C + ct * P:kc * C + (ct + 1) * P],
                            ohT[0:kcnt, kc:kc + 1],
                            start=(kc == 0), stop=(kc == 3))
                wsel = cp.tile([P, CT], bf16)
                nc.vector.tensor_copy(wsel[:], wps[:])

                # cam[1, 784] = w_sel @ x  (center view of padded x)
                cam_ps = [mps.tile([1, NF], f32, tag="mp", name=f"cam_ps{_nh}")
                          for _nh in range(NH)]
                for nh in range(NH):
                    for ct in range(CT):
                        nc.tensor.matmul(
                            cam_ps[nh][:], wsel[:, ct:ct + 1],
                            xview(ct, 4, nh),
                            start=(ct == 0), stop=(ct == CT - 1))
                cam_row = cp.tile([1, PADN], f32)
                for nh in range(NH):
                    nc.vector.tensor_copy(
                        cam_row[0:1, nh * NF:(nh + 1) * NF], cam_ps[nh][:])

                cmin = cp.tile([1, 1], f32)
                cmax = cp.tile([1, 1], f32)
                nc.vector.tensor_reduce(cmin[:], cam_row[0:1, 0:NPOS],
                                        axis=AX, op=OP.min)
                nc.vector.reduce_max(cmax[:], cam_row[0:1, 0:NPOS], axis=AX)
                rng_t = cp.tile([1, 1], f32)
                nc.vector.tensor_scalar(rng_t[:], cmax[:], cmin[:], None,
                                        op0=OP.subtract)
                inv = cp.tile([1, 1], f32)
                nc.vector.reciprocal(inv[:], rng_t[:])

                camn = cp.tile([1, PADN], f32)
                nc.vector.memset(camn[:], NEG)
                nc.vector.tensor_scalar(camn[0:1, 0:NPOS],
                                        cam_row[0:1, 0:NPOS],
                                        cmin[:], inv[:],
                                        op0=OP.subtract, op1=OP.mult)

                # broadcast camn across partitions: camB[128, 784]
                camB = cp.tile([P, NPOS], f32)
                for nh in range(NH):
                    cb_ps = mps.tile([P, NF], f32, tag="mp")
                    nc.tensor.matmul(cb_ps[:], ones_row[:],
                                     camn[0:1, nh * NF:(nh + 1) * NF],
                                     start=True, stop=True)
                    nc.vector.tensor_copy(
                        camB[:, nh * NF:(nh + 1) * NF], cb_ps[:])

                # camn in partition layout [128, 7]
                cnp_ps = mps.tile([P, 7], f32, tag="mp")
                for a in range(7):
                    nc.tensor.transpose(cnp_ps[:, a:a + 1],
                                        camn[0:1, a * P:(a + 1) * P],
                                        ones11[:])
                camnP = cp.tile([P, 7], f32)
                nc.vector.tensor_copy(camnP[:], cnp_ps[:])

                # rank[p,a] = #{j: camn[j] >= camn[p,a]}; top-392 = rank<=392
                ge = cp.tile([P, NPOS], f32)
                rank = cp.tile([P, 7], f32)
                for a in range(7):
                    nc.vector.tensor_scalar(ge[:], camB[:],
                                            camnP[:, a:a + 1],
                                            None, op0=OP.is_ge, op1=OP.add,
                                            accum_out=rank[:, a:a + 1])
                maskP = cp.tile([P, 7], f32)
                nc.vector.tensor_scalar(maskP[:], rank[:], float(N_TOKEN),
                                        None, op0=OP.is_le)
                yP = cp.tile([P, 7], f32)
                nc.vector.tensor_mul(yP[:], maskP[:], camnP[:])
                fe["yP"] = yP

            # ---------- conv main loop ----------
            s_ps = [sps.tile([1, NF], f32, tag=f"s{nh}", name=f"s_ps{nh}")
                    for nh in range(NH)]

            def emit_conv_dt(dt):
                ps = [cps.tile([P, NF], f32, tag=f"cv{nh}",
                               name=f"ps{dt}_{nh}")
                      for nh in range(NH)]
                if dt == 0:
                    wtile = None
                else:
                    wtile = wp.tile([P, CT * 9 * P], bf16, name="w_big",
                                    tag="w_big")
                    nc.sync.dma_start(wtile[:], wt_d[dt])
                for ct in range(CT):
                    if dt == 0:
                        w_ct = wp.tile([P, 9 * P], bf16, name="w_ct",
                                       tag="w_ct")
                        nc.sync.dma_start(
                            w_ct[:],
                            wt_d[dt][:, ct * 9 * P:(ct + 1) * 9 * P])
                        nc.gpsimd.dma_start(xtiles[ct][:], xpad_d[ct])
                    for tap in range(9):
                        if dt == 0:
                            lhsT = w_ct[:, tap * P:(tap + 1) * P]
                        else:
                            lhsT = wtile[:, ct * 9 * P + tap * P:
                                         ct * 9 * P + (tap + 1) * P]
                        for nh in range(NH):
                            nc.tensor.matmul(
                                ps[nh][:], lhsT, xview(ct, tap, nh),
                                start=(ct == 0 and tap == 0),
                                stop=(ct == CT - 1 and tap == 8))
                for nh in range(NH):
                    relu_t = rp.tile([P, NF], bf16, name="relu_t")
                    nc.scalar.activation(relu_t[:], ps[nh][:], AF.Relu,
                                         bias=cb_sb[:, dt:dt + 1])
                    nc.tensor.matmul(s_ps[nh][:], sw_sb[:, dt:dt + 1],
                                     relu_t[:],
                                     start=(dt == 0), stop=(dt == DT - 1))

            emit_conv_dt(0)
            emit_frontend()
            yP = fe["yP"]
            for _dt in range(1, DT):
                emit_conv_dt(_dt)

            # ---------- epilogue: BCE = sum softplus(xcam) - sum xcam*y ----
            xcam_row = cp.tile([1, PADN], f32)
            nc.vector.memset(xcam_row[:], 0.0)
            for nh in range(NH):
                nc.vector.tensor_scalar(
                    xcam_row[0:1, nh * NF:(nh + 1) * NF], s_ps[nh][:],
                    sb_sb[:], None, op0=OP.add)

            et = cp.tile([1, NPOS], f32)
            nc.scalar.activation(et[:], xcam_row[0:1, 0:NPOS], AF.Exp)
            sp = cp.tile([1, NPOS], f32)
            sp_sum = cp.tile([1, 1], f32)
            nc.scalar.activation(sp[:], et[:], AF.Ln, bias=1.0,
                                 accum_out=sp_sum[:])

            xcp_ps = mps.tile([P, 7], f32, tag="mp")
            for a in range(7):
                nc.tensor.transpose(xcp_ps[:, a:a + 1],
                                    xcam_row[0:1, a * P:(a + 1) * P],
                                    ones11[:])
            xcamP = cp.tile([P, 7], f32)
            nc.vector.tensor_copy(xcamP[:], xcp_ps[:])

            prodP = cp.tile([P, 7], f32)
            nc.vector.tensor_mul(prodP[:], yP[:], xcamP[:])
            partial = cp.tile([P, 1], f32)
            nc.vector.reduce_sum(partial[:], prodP[:], axis=AX)

            dot_ps = mps.tile([1, 1], f32, tag="mp")
            nc.tensor.matmul(dot_ps[:], ones_col[:], partial[:],
                             start=True, stop=True)

            final = cp.tile([1, 1], f32)
            nc.vector.tensor_scalar(final[:], dot_ps[:], -1.0, sp_sum[:],
                                    op0=OP.mult, op1=OP.add)
            nc.sync.dma_start(out_d[:], final[:])

    nc.compile()
    return nc


def _prep_in_maps(x, x_fpv_pred, proj_weight, conv1_w, conv1_b, score_w,
                  score_b):
    bf16 = ml_dtypes.bfloat16
    xr = np.asarray(x, np.float32).reshape(B, CT, P, T, H, W)
    xpad = np.zeros((B, CT, P, T, 9, 9), dtype=bf16)
    xpad[:, :, :, :, 1:8, 1:8] = xr.astype(bf16)
    xpad = np.ascontiguousarray(xpad.reshape(B, CT, P, T * PLANE))

    w9 = np.asarray(conv1_w, np.float32).reshape(D, C, 9)
    # wt[dt, p, ct*9*P + tap*P + q] = conv1_w[dt*P+q, ct*P+p, tap]
    wt = np.ascontiguousarray(
        w9.reshape(DT, P, CT, P, 9).transpose(0, 3, 2, 4, 1)
        .reshape(DT, P, CT * 9 * P)).astype(bf16)

    proj_bf = np.asarray(proj_weight, np.float32).astype(bf16)
    cb = np.ascontiguousarray(
        np.asarray(conv1_b, np.float32).reshape(DT, P).T)
    sw = np.ascontiguousarray(
        np.asarray(score_w, np.float32).reshape(DT, P).T).astype(bf16)
    sb = np.asarray(score_b, np.float32).reshape(1, 1)
    xfp = np.asarray(x_fpv_pred, np.float32)

    in_maps = []
    for b in range(B):
        in_maps.append({
            "xpad": xpad[b],
            "wt": wt,
            "proj": proj_bf,
            "xfp": np.ascontiguousarray(xfp[b:b + 1]),
            "cb": cb,
            "sw": sw,
            "sb": sb,
        })
    return in_maps


def run(inputs, trace=False):
    """Build (cached), run on 8 cores, return (loss, BassKernelResults)."""
    from concourse.bass_utils import run_bass_kernel_spmd

    if "nc" not in _cache:
        _cache["nc"] = _build_nc()
    nc = _cache["nc"]
    in_maps = _prep_in_maps(**inputs)
    res = run_bass_kernel_spmd(nc, in_maps, core_ids=list(range(B)),
                               trace=trace)
    total = sum(float(np.asarray(res.results[b]["out"])[0, 0])
                for b in range(B))
    loss = np.float32(total / float(B * T * H * W))
    return loss, res


def kernel(**inputs):
    loss, _ = run(inputs, trace=False)
    return loss


# revision 7
# speedup vs baseline: 1.0853x; 1.0853x over previous
"""Trainium2 Bass kernel for nn_CAMLocalHead (CAM target + conv head + BCE).

Self-contained: takes FULL inputs, shards batch B=8 across 8 NeuronCores
(one sample per core), runs a Bass/Tile kernel per core, sums the per-core
partial BCE sums on host.
"""
import sys

for _p in ("/opt/trn_rl_repo", "/opt/pypackages"):
    if _p not in sys.path:
        sys.path.append(_p)

import numpy as np
import ml_dtypes

# Problem dims (hardcoded per spec)
B, C, T, H, W = 8, 2048, 16, 7, 7
K, D = 400, 512
N_TOKEN = 392
P = 128
CT = C // P          # 16 c-tiles
DT = D // P          # 4 d-tiles
NH = 2               # spatial halves (t 0..7, 8..15)
TH = T // NH         # 8
PLANE = 81           # 9x9 padded plane
NF = TH * H * W      # 392 positions per half
NPOS = T * H * W     # 784
PADN = 7 * P         # 896 (784 padded to 7 chunks of 128)
NEG = -1.0e30

_cache = {}


def _build_nc():
    import concourse.bacc as bacc
    import concourse.mybir as mybir
    from concourse import tile

    f32 = mybir.dt.float32
    bf16 = mybir.dt.bfloat16
    AX = mybir.AxisListType.X
    OP = mybir.AluOpType
    AF = mybir.ActivationFunctionType

    nc = bacc.Bacc(trn_type="TRN2")

    xpad_d = nc.dram_tensor("xpad", [CT, P, T * PLANE], bf16, kind="ExternalInput")
    wt_d = nc.dram_tensor("wt", [DT, P, CT * 9 * P], bf16, kind="ExternalInput")
    proj_d = nc.dram_tensor("proj", [K, C], bf16, kind="ExternalInput")
    xfp_d = nc.dram_tensor("xfp", [1, K], f32, kind="ExternalInput")
    cb_d = nc.dram_tensor("cb", [P, DT], f32, kind="ExternalInput")
    sw_d = nc.dram_tensor("sw", [P, DT], bf16, kind="ExternalInput")
    sb_d = nc.dram_tensor("sb", [1, 1], f32, kind="ExternalInput")
    out_d = nc.dram_tensor("out", [1, 1], f32, kind="ExternalOutput")

    with tile.TileContext(nc) as tc:
        with (
            tc.tile_pool(name="const", bufs=1) as cp,
            tc.tile_pool(name="wp", bufs=2) as wp,
            tc.tile_pool(name="rp", bufs=4) as rp,
            tc.tile_pool(name="cps", bufs=2, space="PSUM") as cps,
            tc.tile_pool(name="sps", bufs=1, space="PSUM") as sps,
            tc.tile_pool(name="mps", bufs=2, space="PSUM") as mps,
        ):
            # ---------- constants / inputs resident in SBUF ----------
            xtiles = [cp.tile([P, T * PLANE], bf16, name=f"xt{ct}")
                      for ct in range(CT)]

            xfp = cp.tile([1, K], f32)
            nc.scalar.dma_start(xfp[:], xfp_d[:])
            cb_sb = cp.tile([P, DT], f32)
            nc.scalar.dma_start(cb_sb[:], cb_d[:])
            sw_sb = cp.tile([P, DT], bf16)
            nc.scalar.dma_start(sw_sb[:], sw_d[:])
            sb_sb = cp.tile([1, 1], f32)
            nc.scalar.dma_start(sb_sb[:], sb_d[:])

            ones11 = cp.tile([1, 1], f32)
            nc.vector.memset(ones11[:], 1.0)
            ones_row = cp.tile([1, P], f32)
            nc.vector.memset(ones_row[:], 1.0)
            ones_col = cp.tile([P, 1], f32)
            nc.vector.memset(ones_col[:], 1.0)

            def xview(ct, tap, nh):
                dh, dw = tap // 3, tap % 3
                v = xtiles[ct][:].rearrange(
                    "p (t h w) -> p t h w", t=T, h=9, w=9)
                return v[:, nh * TH:(nh + 1) * TH, dh:dh + 7, dw:dw + 7]

            # ---------- CAM front-end (emitted between conv dt0 and dt1
            # so the DMA preamble overlaps PE work) ----------
            fe = {}

            def emit_frontend():
                proj_sb = cp.tile([P, 4 * C], bf16)
                for kc in range(4):
                    kcnt = min(P, K - kc * P)
                    nc.scalar.dma_start(
                        proj_sb[0:kcnt, kc * C:(kc + 1) * C],
                        proj_d[kc * P:kc * P + kcnt, :])
                # argmax class via one-hot (sigmoid monotonic -> argmax on raw)
                m = cp.tile([1, 1], f32)
                nc.vector.reduce_max(m[:], xfp[:], axis=AX)
                oh = cp.tile([1, 4 * P], f32)
                nc.vector.memset(oh[:], 0.0)
                nc.vector.tensor_scalar(oh[0:1, 0:K], xfp[:], m[:], None,
                                        op0=OP.is_equal)
                ohT_ps = mps.tile([P, 4], f32, tag="mp")
                for i in range(4):
                    nc.tensor.transpose(ohT_ps[:, i:i + 1],
                                        oh[0:1, i * P:(i + 1) * P], ones11[:])
                ohT = cp.tile([P, 4], bf16)
                nc.vector.tensor_copy(ohT[:], ohT_ps[:])

                # w_selT[c] = proj_weight[top_cls, c], [128, CT] (c-tile cols)
                wps = mps.tile([P, CT], f32, tag="mp")
                for ct in range(CT):
                    for kc in range(4):
                        kcnt = min(P, K - kc * P)
                        nc.tensor.matmul(
                            wps[:, ct:ct + 1],
                            proj_sb[0:kcnt,
                                    kc * C + ct * P:kc * C + (ct + 1) * P],
                            ohT[0:kcnt, kc:kc + 1],
                            start=(kc == 0), stop=(kc == 3))
                wsel = cp.tile([P, CT], bf16)
                nc.vector.tensor_copy(wsel[:], wps[:])

                # cam[1, 784] = w_sel @ x  (center view of padded x)
                cam_ps = [mps.tile([1, NF], f32, tag="mp", name=f"cam_ps{_nh}")
                          for _nh in range(NH)]
                for nh in range(NH):
                    for ct in range(CT):
                        nc.tensor.matmul(
                            cam_ps[nh][:], wsel[:, ct:ct + 1],
                            xview(ct, 4, nh),
                            start=(ct == 0), stop=(ct == CT - 1))
                cam_row = cp.tile([1, PADN], f32)
                for nh in range(NH):
                    nc.vector.tensor_copy(
                        cam_row[0:1, nh * NF:(nh + 1) * NF], cam_ps[nh][:])

                cmin = cp.tile([1, 1], f32)
                cmax = cp.tile([1, 1], f32)
                nc.vector.tensor_reduce(cmin[:], cam_row[0:1, 0:NPOS],
                                        axis=AX, op=OP.min)
                nc.vector.reduce_max(cmax[:], cam_row[0:1, 0:NPOS], axis=AX)
                rng_t = cp.tile([1, 1], f32)
                nc.vector.tensor_scalar(rng_t[:], cmax[:], cmin[:], None,
                                        op0=OP.subtract)
                inv = cp.tile([1, 1], f32)
                nc.vector.reciprocal(inv[:], rng_t[:])

                camn = cp.tile([1, PADN], f32)
                nc.vector.memset(camn[:], NEG)
                nc.vector.tensor_scalar(camn[0:1, 0:NPOS],
                                        cam_row[0:1, 0:NPOS],
                                        cmin[:], inv[:],
                                        op0=OP.subtract, op1=OP.mult)

                # broadcast camn across partitions: camB[128, 784]
                camB = cp.tile([P, NPOS], f32)
                for nh in range(NH):
                    cb_ps = mps.tile([P, NF], f32, tag="mp")
                    nc.tensor.matmul(cb_ps[:], ones_row[:],
                                     camn[0:1, nh * NF:(nh + 1) * NF],
                                     start=True, stop=True)
                    nc.vector.tensor_copy(
                        camB[:, nh * NF:(nh + 1) * NF], cb_ps[:])

                # camn in partition layout [128, 7]
                cnp_ps = mps.tile([P, 7], f32, tag="mp")
                for a in range(7):
                    nc.tensor.transpose(cnp_ps[:, a:a + 1],
                                        camn[0:1, a * P:(a + 1) * P],
                                        ones11[:])
                camnP = cp.tile([P, 7], f32)
                nc.vector.tensor_copy(camnP[:], cnp_ps[:])

                # rank[p,a] = #{j: camn[j] >= camn[p,a]}; top-392 = rank<=392
                ge = cp.tile([P, NPOS], f32)
                rank = cp.tile([P, 7], f32)
                for a in range(7):
                    nc.vector.tensor_scalar(ge[:], camB[:],
                                            camnP[:, a:a + 1],
                                            None, op0=OP.is_ge, op1=OP.add,
                                            accum_out=rank[:, a:a + 1])
                maskP = cp.tile([P, 7], f32)
                nc.vector.tensor_scalar(maskP[:], rank[:], float(N_TOKEN),
                                        None, op0=OP.is_le)
                yP = cp.tile([P, 7], f32)
                nc.vector.tensor_mul(yP[:], maskP[:], camnP[:])
                fe["yP"] = yP

            # ---------- conv main loop ----------
            s_ps = [sps.tile([1, NF], f32, tag=f"s{nh}", name=f"s_ps{nh}")
                    for nh in range(NH)]

            def emit_conv_dt(dt):
                ps = [cps.tile([P, NF], f32, tag=f"cv{nh}",
                               name=f"ps{dt}_{nh}")
                      for nh in range(NH)]
                if dt == 0:
                    wtile = None
                else:
                    wtile = wp.tile([P, CT * 9 * P], bf16, name="w_big",
                                    tag="w_big")
                    nc.sync.dma_start(wtile[:], wt_d[dt])
                for ct in range(CT):
                    if dt == 0:
                        w_ct = wp.tile([P, 9 * P], bf16, name="w_ct",
                                       tag="w_ct")
                        nc.sync.dma_start(
                            w_ct[:],
                            wt_d[dt][:, ct * 9 * P:(ct + 1) * 9 * P])
                        nc.sync.dma_start(xtiles[ct][:], xpad_d[ct])
                    for tap in range(9):
                        if dt == 0:
                            lhsT = w_ct[:, tap * P:(tap + 1) * P]
                        else:
                            lhsT = wtile[:, ct * 9 * P + tap * P:
                                         ct * 9 * P + (tap + 1) * P]
                        for nh in range(NH):
                            nc.tensor.matmul(
                                ps[nh][:], lhsT, xview(ct, tap, nh),
                                start=(ct == 0 and tap == 0),
                                stop=(ct == CT - 1 and tap == 8))
                for nh in range(NH):
                    relu_t = rp.tile([P, NF], bf16, name="relu_t")
                    nc.scalar.activation(relu_t[:], ps[nh][:], AF.Relu,
                                         bias=cb_sb[:, dt:dt + 1])
                    nc.tensor.matmul(s_ps[nh][:], sw_sb[:, dt:dt + 1],
                                     relu_t[:],
                                     start=(dt == 0), stop=(dt == DT - 1))

            emit_conv_dt(0)
            emit_conv_dt(1)
            emit_frontend()
            yP = fe["yP"]
            emit_conv_dt(2)
            emit_conv_dt(3)

            # ---------- epilogue: BCE = sum softplus(xcam) - sum xcam*y ----
            xcam_row = cp.tile([1, PADN], f32)
            nc.vector.memset(xcam_row[:], 0.0)
            for nh in range(NH):
                nc.vector.tensor_scalar(
                    xcam_row[0:1, nh * NF:(nh + 1) * NF], s_ps[nh][:],
                    sb_sb[:], None, op0=OP.add)

            et = cp.tile([1, NPOS], f32)
            nc.scalar.activation(et[:], xcam_row[0:1, 0:NPOS], AF.Exp)
            sp = cp.tile([1, NPOS], f32)
            sp_sum = cp.tile([1, 1], f32)
            nc.scalar.activation(sp[:], et[:], AF.Ln, bias=1.0,
                                 accum_out=sp_sum[:])

            xcp_ps = mps.tile([P, 7], f32, tag="mp")
            for a in range(7):
                nc.tensor.transpose(xcp_ps[:, a:a + 1],
                                    xcam_row[0:1, a * P:(a + 1) * P],
                                    ones11[:])
            xcamP = cp.tile([P, 7], f32)
            nc.vector.tensor_copy(xcamP[:], xcp_ps[:])

            prodP = cp.tile([P, 7], f32)
            nc.vector.tensor_mul(prodP[:], yP[:], xcamP[:])
            partial = cp.tile([P, 1], f32)
            nc.vector.reduce_sum(partial[:], prodP[:], axis=AX)

            dot_ps = mps.tile([1, 1], f32, tag="mp")
            nc.tensor.matmul(dot_ps[:], ones_col[:], partial[:],
                             start=True, stop=True)

            final = cp.tile([1, 1], f32)
            nc.vector.tensor_scalar(final[:], dot_ps[:], -1.0, sp_sum[:],
                                    op0=OP.mult, op1=OP.add)
            nc.sync.dma_start(out_d[:], final[:])

    nc.compile()
    return nc


def _prep_in_maps(x, x_fpv_pred, proj_weight, conv1_w, conv1_b, score_w,
                  score_b):
    bf16 = ml_dtypes.bfloat16
    xr = np.asarray(x, np.float32).reshape(B, CT, P, T, H, W)
    xpad = np.zeros((B, CT, P, T, 9, 9), dtype=bf16)
    xpad[:, :, :, :, 1:8, 1:8] = xr.astype(bf16)
    xpad = np.ascontiguousarray(xpad.reshape(B, CT, P, T * PLANE))

    w9 = np.asarray(conv1_w, np.float32).reshape(D, C, 9)
    # wt[dt, p, ct*9*P + tap*P + q] = conv1_w[dt*P+q, ct*P+p, tap]
    wt = np.ascontiguousarray(
        w9.reshape(DT, P, CT, P, 9).transpose(0, 3, 2, 4, 1)
        .reshape(DT, P, CT * 9 * P)).astype(bf16)

    proj_bf = np.asarray(proj_weight, np.float32).astype(bf16)
    cb = np.ascontiguousarray(
        np.asarray(conv1_b, np.float32).reshape(DT, P).T)
    sw = np.ascontiguousarray(
        np.asarray(score_w, np.float32).reshape(DT, P).T).astype(bf16)
    sb = np.asarray(score_b, np.float32).reshape(1, 1)
    xfp = np.asarray(x_fpv_pred, np.float32)

    in_maps = []
    for b in range(B):
        in_maps.append({
            "xpad": xpad[b],
            "wt": wt,
            "proj": proj_bf,
            "xfp": np.ascontiguousarray(xfp[b:b + 1]),
            "cb": cb,
            "sw": sw,
            "sb": sb,
        })
    return in_maps


def run(inputs, trace=False):
    """Build (cached), run on 8 cores, return (loss, BassKernelResults)."""
    from concourse.bass_utils import run_bass_kernel_spmd

    if "nc" not in _cache:
        _cache["nc"] = _build_nc()
    nc = _cache["nc"]
    in_maps = _prep_in_maps(**inputs)
    res = run_bass_kernel_spmd(nc, in_maps, core_ids=list(range(B)),
                               trace=trace)
    total = sum(float(np.asarray(res.results[b]["out"])[0, 0])
                for b in range(B))
    loss = np.float32(total / float(B * T * H * W))
    return loss, res


def kernel(**inputs):
    loss, _ = run(inputs, trace=False)
    return loss


# revision 10
# speedup vs baseline: 1.7155x; 1.5807x over previous
"""Trainium2 Bass kernel for nn_CAMLocalHead (CAM target + conv head + BCE).

Self-contained: takes FULL inputs, shards batch B=8 across 8 NeuronCores
(one sample per core), runs a Bass/Tile kernel per core, sums the per-core
partial BCE sums on host.
"""
import sys

for _p in ("/opt/trn_rl_repo", "/opt/pypackages"):
    if _p not in sys.path:
        sys.path.append(_p)

import numpy as np
import ml_dtypes

# Problem dims (hardcoded per spec)
B, C, T, H, W = 8, 2048, 16, 7, 7
K, D = 400, 512
N_TOKEN = 392
P = 128
CT = C // P          # 16 c-tiles
DT = D // P          # 4 d-tiles
NH = 2               # spatial halves (t 0..7, 8..15)
TH = T // NH         # 8
PLANE = 81           # 9x9 padded plane
NF = TH * H * W      # 392 positions per half
NPOS = T * H * W     # 784
PADN = 7 * P         # 896 (784 padded to 7 chunks of 128)
NEG = -1.0e30

_cache = {}


def _build_nc():
    import concourse.bacc as bacc
    import concourse.mybir as mybir
    from concourse import tile

    f32 = mybir.dt.float32
    bf16 = mybir.dt.bfloat16
    fp8 = mybir.dt.float8e4
    DR = mybir.MatmulPerfMode.DoubleRow
    AX = mybir.AxisListType.X
    OP = mybir.AluOpType
    AF = mybir.ActivationFunctionType

    nc = bacc.Bacc(trn_type="TRN2")

    xpad_d = nc.dram_tensor("xpad", [CT, P, T * PLANE], bf16, kind="ExternalInput")
    w8_d = nc.dram_tensor("w8", [DT, P, 8 * 9 * 2 * P], fp8, kind="ExternalInput")
    xp8_d = nc.dram_tensor("xp8", [8, P, 2 * T * 96], fp8, kind="ExternalInput")
    proj_d = nc.dram_tensor("proj", [K, C], bf16, kind="ExternalInput")
    xfp_d = nc.dram_tensor("xfp", [1, K], f32, kind="ExternalInput")
    cb_d = nc.dram_tensor("cb", [P, DT], f32, kind="ExternalInput")
    sw_d = nc.dram_tensor("sw", [P, DT], bf16, kind="ExternalInput")
    sb_d = nc.dram_tensor("sb", [1, 1], f32, kind="ExternalInput")
    out_d = nc.dram_tensor("out", [1, 1], f32, kind="ExternalOutput")

    with tile.TileContext(nc) as tc:
        with (
            tc.tile_pool(name="const", bufs=1) as cp,
            tc.tile_pool(name="wp", bufs=2) as wp,
            tc.tile_pool(name="rp", bufs=4) as rp,
            tc.tile_pool(name="cps", bufs=2, space="PSUM") as cps,
            tc.tile_pool(name="sps", bufs=1, space="PSUM") as sps,
            tc.tile_pool(name="mps", bufs=2, space="PSUM") as mps,
        ):
            # ---------- constants / inputs resident in SBUF ----------
            xtiles = []
            for ct in range(CT):
                xt = cp.tile([P, T * PLANE], bf16, name=f"xt{ct}")
                nc.gpsimd.dma_start(xt[:], xpad_d[ct])
                xtiles.append(xt)

            xp8tiles = [cp.tile([P, 2 * T * 96], fp8, name=f"xp8_{i}")
                        for i in range(8)]

            xfp = cp.tile([1, K], f32)
            nc.scalar.dma_start(xfp[:], xfp_d[:])
            cb_sb = cp.tile([P, DT], f32)
            nc.scalar.dma_start(cb_sb[:], cb_d[:])
            sw_sb = cp.tile([P, DT], bf16)
            nc.scalar.dma_start(sw_sb[:], sw_d[:])
            sb_sb = cp.tile([1, 1], f32)
            nc.scalar.dma_start(sb_sb[:], sb_d[:])

            ones11 = cp.tile([1, 1], f32)
            nc.vector.memset(ones11[:], 1.0)
            ones_row = cp.tile([1, P], f32)
            nc.vector.memset(ones_row[:], 1.0)
            ones_col = cp.tile([P, 1], f32)
            nc.vector.memset(ones_col[:], 1.0)

            def xview(ct, tap, nh):
                dh, dw = tap // 3, tap % 3
                v = xtiles[ct][:].rearrange(
                    "p (t h w) -> p t h w", t=T, h=9, w=9)
                return v[:, nh * TH:(nh + 1) * TH, dh:dh + 7, dw:dw + 7]

            # ---------- CAM front-end (emitted between conv dt0 and dt1
            # so the DMA preamble overlaps PE work) ----------
            fe = {}

            def emit_frontend():
                proj_sb = cp.tile([P, 4 * C], bf16)
                for kc in range(4):
                    kcnt = min(P, K - kc * P)
                    nc.scalar.dma_start(
                        proj_sb[0:kcnt, kc * C:(kc + 1) * C],
                        proj_d[kc * P:kc * P + kcnt, :])
                # argmax class via one-hot (sigmoid monotonic -> argmax on raw)
                m = cp.tile([1, 1], f32)
                nc.vector.reduce_max(m[:], xfp[:], axis=AX)
                oh = cp.tile([1, 4 * P], f32)
                nc.vector.memset(oh[:], 0.0)
                nc.vector.tensor_scalar(oh[0:1, 0:K], xfp[:], m[:], None,
                                        op0=OP.is_equal)
                ohT_ps = mps.tile([P, 4], f32, tag="mp")
                for i in range(4):
                    nc.tensor.transpose(ohT_ps[:, i:i + 1],
                                        oh[0:1, i * P:(i + 1) * P], ones11[:])
                ohT = cp.tile([P, 4], bf16)
                nc.vector.tensor_copy(ohT[:], ohT_ps[:])

                # w_selT[c] = proj_weight[top_cls, c], [128, CT] (c-tile cols)
                wps = mps.tile([P, CT], f32, tag="mp")
                for ct in range(CT):
                    for kc in range(4):
                        kcnt = min(P, K - kc * P)
                        nc.tensor.matmul(
                            wps[:, ct:ct + 1],
                            proj_sb[0:kcnt,
                                    kc * C + ct * P:kc * C + (ct + 1) * P],
                            ohT[0:kcnt, kc:kc + 1],
                            start=(kc == 0), stop=(kc == 3))
                wsel = cp.tile([P, CT], bf16)
                nc.vector.tensor_copy(wsel[:], wps[:])

                # cam[1, 784] = w_sel @ x  (center view of padded x)
                cam_ps = [mps.tile([1, NF], f32, tag="mp", name=f"cam_ps{_nh}")
                          for _nh in range(NH)]
                for nh in range(NH):
                    for ct in range(CT):
                        nc.tensor.matmul(
                            cam_ps[nh][:], wsel[:, ct:ct + 1],
                            xview(ct, 4, nh),
                            start=(ct == 0), stop=(ct == CT - 1))
                cam_row = cp.tile([1, PADN], f32)
                for nh in range(NH):
                    nc.vector.tensor_copy(
                        cam_row[0:1, nh * NF:(nh + 1) * NF], cam_ps[nh][:])

                cmin = cp.tile([1, 1], f32)
                cmax = cp.tile([1, 1], f32)
                nc.vector.tensor_reduce(cmin[:], cam_row[0:1, 0:NPOS],
                                        axis=AX, op=OP.min)
                nc.vector.reduce_max(cmax[:], cam_row[0:1, 0:NPOS], axis=AX)
                rng_t = cp.tile([1, 1], f32)
                nc.vector.tensor_scalar(rng_t[:], cmax[:], cmin[:], None,
                                        op0=OP.subtract)
                inv = cp.tile([1, 1], f32)
                nc.vector.reciprocal(inv[:], rng_t[:])

                camn = cp.tile([1, PADN], f32)
                nc.vector.memset(camn[:], NEG)
                nc.vector.tensor_scalar(camn[0:1, 0:NPOS],
                                        cam_row[0:1, 0:NPOS],
                                        cmin[:], inv[:],
                                        op0=OP.subtract, op1=OP.mult)

                # broadcast camn across partitions: camB[128, 784]
                camB = cp.tile([P, NPOS], f32)
                for nh in range(NH):
                    cb_ps = mps.tile([P, NF], f32, tag="mp")
                    nc.tensor.matmul(cb_ps[:], ones_row[:],
                                     camn[0:1, nh * NF:(nh + 1) * NF],
                                     start=True, stop=True)
                    nc.vector.tensor_copy(
                        camB[:, nh * NF:(nh + 1) * NF], cb_ps[:])

                # camn in partition layout [128, 7]
                cnp_ps = mps.tile([P, 7], f32, tag="mp")
                for a in range(7):
                    nc.tensor.transpose(cnp_ps[:, a:a + 1],
                                        camn[0:1, a * P:(a + 1) * P],
                                        ones11[:])
                camnP = cp.tile([P, 7], f32)
                nc.vector.tensor_copy(camnP[:], cnp_ps[:])

                # rank[p,a] = #{j: camn[j] >= camn[p,a]}; top-392 = rank<=392
                ge = cp.tile([P, NPOS], f32)
                rank = cp.tile([P, 7], f32)
                for a in range(7):
                    nc.vector.tensor_scalar(ge[:], camB[:],
                                            camnP[:, a:a + 1],
                                            None, op0=OP.is_ge, op1=OP.add,
                                            accum_out=rank[:, a:a + 1])
                maskP = cp.tile([P, 7], f32)
                nc.vector.tensor_scalar(maskP[:], rank[:], float(N_TOKEN),
                                        None, op0=OP.is_le)
                yP = cp.tile([P, 7], f32)
                nc.vector.tensor_mul(yP[:], maskP[:], camnP[:])
                fe["yP"] = yP

            # ---------- conv main loop ----------
            s_ps = [sps.tile([1, NF], f32, tag=f"s{nh}", name=f"s_ps{nh}")
                    for nh in range(NH)]

            # conv: fp8 DoubleRow over ct-pairs; rhs reads 63-col
            # contiguous runs per t-plane (2 junk cols per row land in
            # unused PSUM columns), out psum [128, 8*63=504] per half.
            CW = 63
            NJ = TH * CW  # 504

            def emit_conv_dt(dt):
                ps = [cps.tile([P, NJ], f32, tag=f"cv{nh}",
                               name=f"ps{dt}_{nh}")
                      for nh in range(NH)]
                if dt == 0:
                    wtile = None
                else:
                    wtile = wp.tile([P, 8 * 9 * 2 * P], fp8, name="w_big",
                                    tag="w_big")
                    nc.sync.dma_start(wtile[:], w8_d[dt])
                for ctp in range(8):
                    if dt == 0:
                        w_ct = wp.tile([P, 9 * 2 * P], fp8, name="w_ct",
                                       tag="w_ct")
                        nc.sync.dma_start(
                            w_ct[:],
                            w8_d[dt][:, ctp * 9 * 2 * P:
                                     (ctp + 1) * 9 * 2 * P])
                        nc.sync.dma_start(xp8tiles[ctp][:], xp8_d[ctp])
                    xpv = xp8tiles[ctp][:].rearrange(
                        "p (two t hw) -> p two t hw", two=2, t=T, hw=96)
                    for tap in range(9):
                        base = (tap // 3) * 9 + tap % 3
                        if dt == 0:
                            wsl = w_ct[:, tap * 2 * P:(tap + 1) * 2 * P]
                        else:
                            wsl = wtile[:, (ctp * 9 + tap) * 2 * P:
                                        (ctp * 9 + tap + 1) * 2 * P]
                        lhsT3 = wsl.rearrange("p (two q) -> p two q", two=2)
                        for nh in range(NH):
                            rhs = xpv[:, :, nh * TH:(nh + 1) * TH,
                                      base:base + CW]
                            nc.tensor.matmul(
                                ps[nh][:], lhsT3, rhs,
                                start=(ctp == 0 and tap == 0),
                                stop=(ctp == 7 and tap == 8),
                                perf_mode=DR)
                for nh in range(NH):
                    relu_t = rp.tile([P, NF], bf16, name="relu_t")
                    pv = ps[nh][:].rearrange("p (t h w) -> p t h w",
                                             t=TH, h=7, w=9)
                    nc.scalar.activation(relu_t[:].rearrange(
                        "p (t h w) -> p t h w", t=TH, h=7, w=7),
                        pv[:, :, :, 0:7], AF.Relu,
                        bias=cb_sb[:, dt:dt + 1], scale=1.0 / 64.0)
                    nc.tensor.matmul(s_ps[nh][:], sw_sb[:, dt:dt + 1],
                                     relu_t[:],
                                     start=(dt == 0), stop=(dt == DT - 1))

            emit_conv_dt(0)
            emit_conv_dt(1)
            emit_frontend()
            yP = fe["yP"]
            emit_conv_dt(2)
            emit_conv_dt(3)

            # ---------- epilogue: BCE = sum softplus(xcam) - sum xcam*y ----
            xcam_row = cp.tile([1, PADN], f32)
            nc.vector.memset(xcam_row[:], 0.0)
            for nh in range(NH):
                nc.vector.tensor_scalar(
                    xcam_row[0:1, nh * NF:(nh + 1) * NF], s_ps[nh][:],
                    sb_sb[:], None, op0=OP.add)

            et = cp.tile([1, NPOS], f32)
            nc.scalar.activation(et[:], xcam_row[0:1, 0:NPOS], AF.Exp)
            sp = cp.tile([1, NPOS], f32)
            sp_sum = cp.tile([1, 1], f32)
            nc.scalar.activation(sp[:], et[:], AF.Ln, bias=1.0,
                                 accum_out=sp_sum[:])

            xcp_ps = mps.tile([P, 7], f32, tag="mp")
            for a in range(7):
                nc.tensor.transpose(xcp_ps[:, a:a + 1],
                                    xcam_row[0:1, a * P:(a + 1) * P],
                                    ones11[:])
            xcamP = cp.tile([P, 7], f32)
            nc.vector.tensor_copy(xcamP[:], xcp_ps[:])

            prodP = cp.tile([P, 7], f32)
            nc.vector.tensor_mul(prodP[:], yP[:], xcamP[:])
            partial = cp.tile([P, 1], f32)
            nc.vector.reduce_sum(partial[:], prodP[:], axis=AX)

            dot_ps = mps.tile([1, 1], f32, tag="mp")
            nc.tensor.matmul(dot_ps[:], ones_col[:], partial[:],
                             start=True, stop=True)

            final = cp.tile([1, 1], f32)
            nc.vector.tensor_scalar(final[:], dot_ps[:], -1.0, sp_sum[:],
                                    op0=OP.mult, op1=OP.add)
            nc.sync.dma_start(out_d[:], final[:])

    nc.compile()
    return nc


def _prep_in_maps(x, x_fpv_pred, proj_weight, conv1_w, conv1_b, score_w,
                  score_b):
    bf16 = ml_dtypes.bfloat16
    import concourse.mybir as mybir
    fp8 = mybir.dt.np(mybir.dt.float8e4)
    xr = np.asarray(x, np.float32).reshape(B, CT, P, T, H, W)
    xpadf = np.zeros((B, CT, P, T, 9, 9), dtype=np.float32)
    xpadf[:, :, :, :, 1:8, 1:8] = xr
    xpadf = xpadf.reshape(B, CT, P, T * PLANE)
    xpad = np.ascontiguousarray(xpadf.astype(bf16))
    # xp8: plane stride padded to 96 so 63-wide tap windows stay in-bounds
    xp96 = np.zeros((B, CT, P, T, 96), np.float32)
    xp96[:, :, :, :, :PLANE] = xpadf.reshape(B, CT, P, T, PLANE)
    xp8 = np.ascontiguousarray(
        xp96.reshape(B, 8, 2, P, T * 96).transpose(0, 1, 3, 2, 4)
        .reshape(B, 8, P, 2 * T * 96)).astype(fp8)

    w9 = np.asarray(conv1_w, np.float32).reshape(D, C, 9)
    # w8[dt, p, ((ctp*9 + tap)*2 + two)*P + q]
    #   = 64 * conv1_w[dt*P+q, (2*ctp+two)*P+p, tap]
    w8 = np.ascontiguousarray(
        (w9 * 64.0).reshape(DT, P, 8, 2, P, 9).transpose(0, 4, 2, 5, 3, 1)
        .reshape(DT, P, 8 * 9 * 2 * P)).astype(fp8)

    proj_bf = np.asarray(proj_weight, np.float32).astype(bf16)
    cb = np.ascontiguousarray(
        np.asarray(conv1_b, np.float32).reshape(DT, P).T)
    sw = np.ascontiguousarray(
        np.asarray(score_w, np.float32).reshape(DT, P).T).astype(bf16)
    sb = np.asarray(score_b, np.float32).reshape(1, 1)
    xfp = np.asarray(x_fpv_pred, np.float32)

    in_maps = []
    for b in range(B):
        in_maps.append({
            "xpad": xpad[b],
            "xp8": xp8[b],
            "w8": w8,
            "proj": proj_bf,
            "xfp": np.ascontiguousarray(xfp[b:b + 1]),
            "cb": cb,
            "sw": sw,
            "sb": sb,
        })
    return in_maps


def run(inputs, trace=False):
    """Build (cached), run on 8 cores, return (loss, BassKernelResults)."""
    from concourse.bass_utils import run_bass_kernel_spmd

    if "nc" not in _cache:
        _cache["nc"] = _build_nc()
    nc = _cache["nc"]
    in_maps = _prep_in_maps(**inputs)
    res = run_bass_kernel_spmd(nc, in_maps, core_ids=list(range(B)),
                               trace=trace)
    total = sum(float(np.asarray(res.results[b]["out"])[0, 0])
                for b in range(B))
    loss = np.float32(total / float(B * T * H * W))
    return loss, res


def kernel(**inputs):
    loss, _ = run(inputs, trace=False)
    return loss


# revision 11
# speedup vs baseline: 2.0703x; 1.2068x over previous
"""Trainium2 Bass kernel for nn_CAMLocalHead (CAM target + conv head + BCE).

Self-contained: takes FULL inputs, shards batch B=8 across 8 NeuronCores
(one sample per core), runs a Bass/Tile kernel per core, sums the per-core
partial BCE sums on host.

Device algorithm per core (one sample):
  - argmax class via one-hot (sigmoid is monotonic), selected proj row via
    PE matmuls, CAM = row @ x as fp8 DoubleRow matmuls (scale-invariant).
  - top-392-of-784 mask via rank trick: rank(v) = #{j: cam_j >= v} <= 392,
    computed with a PE broadcast + DVE is_ge accumulations (no sort).
  - Conv3d(2048->512, 1x3x3, pad 011) as 9 shifted fp8 DoubleRow matmuls
    accumulating in PSUM; x stored as 3 w-shifted padded copies so each
    tap reads contiguous 49-element runs per t-plane (no junk columns).
    Weights pre-scaled x64 into e4m3 range; un-scaled via ReLU activation
    scale=1/64. ReLU+bias fused on ACT; score conv = one more matmul per
    d-tile accumulating into a [1, 392] psum.
  - BCE sum = sum ln(1+e^x) - sum x*y  (softplus via Exp then Ln(1+e)).
"""
import sys

for _p in ("/opt/trn_rl_repo", "/opt/pypackages"):
    if _p not in sys.path:
        sys.path.append(_p)

import numpy as np
import ml_dtypes

# Problem dims (hardcoded per spec)
B, C, T, H, W = 8, 2048, 16, 7, 7
K, D = 400, 512
N_TOKEN = 392
P = 128
CT = C // P          # 16 c-tiles
CTP = CT // 2        # 8 c-tile pairs (DoubleRow)
DT = D // P          # 4 d-tiles
NH = 2               # spatial halves (t 0..7, 8..15)
TH = T // NH         # 8
NF = TH * H * W      # 392 positions per half
NPOS = T * H * W     # 784
PADN = 7 * P         # 896 (784 padded to 7 chunks of 128)
NEG = -1.0e30
SHW = 9 * 7          # 63: one w-shifted padded plane (9 rows x 7 cols)
SPT = T * SHW        # 1008: one shift-copy, all t
XF = 2 * 3 * SPT     # 6048: free size of one fp8 x pair-tile

_cache = {}


def _build_nc():
    import concourse.bacc as bacc
    import concourse.mybir as mybir
    from concourse import tile

    f32 = mybir.dt.float32
    bf16 = mybir.dt.bfloat16
    fp8 = mybir.dt.float8e4
    DR = mybir.MatmulPerfMode.DoubleRow
    AX = mybir.AxisListType.X
    OP = mybir.AluOpType
    AF = mybir.ActivationFunctionType

    nc = bacc.Bacc(trn_type="TRN2")

    w8_d = nc.dram_tensor("w8", [DT, P, CTP * 9 * 2 * P], fp8,
                          kind="ExternalInput")
    xp8_d = nc.dram_tensor("xp8", [CTP, P, XF], fp8, kind="ExternalInput")
    proj_d = nc.dram_tensor("proj", [K, C], bf16, kind="ExternalInput")
    xfp_d = nc.dram_tensor("xfp", [1, K], f32, kind="ExternalInput")
    cb_d = nc.dram_tensor("cb", [P, DT], f32, kind="ExternalInput")
    sw_d = nc.dram_tensor("sw", [P, DT], bf16, kind="ExternalInput")
    sb_d = nc.dram_tensor("sb", [1, 1], f32, kind="ExternalInput")
    out_d = nc.dram_tensor("out", [1, 1], f32, kind="ExternalOutput")

    with tile.TileContext(nc) as tc:
        with (
            tc.tile_pool(name="const", bufs=1) as cp,
            tc.tile_pool(name="wps_", bufs=4) as wp,
            tc.tile_pool(name="wpb_", bufs=2) as wpb,
            tc.tile_pool(name="rp", bufs=4) as rp,
            tc.tile_pool(name="cps", bufs=2, space="PSUM") as cps,
            tc.tile_pool(name="sps", bufs=1, space="PSUM") as sps,
            tc.tile_pool(name="mps", bufs=2, space="PSUM") as mps,
        ):
            # ---------- small constants (scalar HWDGE ring) ----------
            xfp = cp.tile([1, K], f32)
            nc.scalar.dma_start(xfp[:], xfp_d[:])
            proj_sb = cp.tile([P, 4 * C], bf16)
            for kc in range(4):
                kcnt = min(P, K - kc * P)
                nc.scalar.dma_start(
                    proj_sb[0:kcnt, kc * C:(kc + 1) * C],
                    proj_d[kc * P:kc * P + kcnt, :])
            cb_sb = cp.tile([P, DT], f32)
            nc.scalar.dma_start(cb_sb[:], cb_d[:])
            sw_sb = cp.tile([P, DT], bf16)
            nc.scalar.dma_start(sw_sb[:], sw_d[:])
            sb_sb = cp.tile([1, 1], f32)
            nc.scalar.dma_start(sb_sb[:], sb_d[:])

            ones11 = cp.tile([1, 1], f32)
            nc.vector.memset(ones11[:], 1.0)
            ones_row = cp.tile([1, P], f32)
            nc.vector.memset(ones_row[:], 1.0)
            ones_col = cp.tile([P, 1], f32)
            nc.vector.memset(ones_col[:], 1.0)

            xp8tiles = [cp.tile([P, XF], fp8, name=f"xp8_{i}")
                        for i in range(CTP)]

            def xp8view(ctp):
                # [p, two, s, t, f63]
                return xp8tiles[ctp][:].rearrange(
                    "p (two s t f) -> p two s t f", two=2, s=3, t=T, f=SHW)

            def conv_rhs(ctp, tap, nh):
                dh, dw = tap // 3, tap % 3
                v = xp8view(ctp)[:, :, dw, nh * TH:(nh + 1) * TH,
                                 dh * 7:dh * 7 + 49]
                return v  # [p, 2, TH, 49] -> free 784, halved by DoubleRow

            # ---------- CAM front-end (emitted between conv dt1 and dt2
            # so its DMA/DVE deps resolve while PE chews on conv) --------
            fe = {}

            def emit_frontend():
                # argmax class via one-hot (sigmoid monotonic)
                m = cp.tile([1, 1], f32)
                nc.vector.reduce_max(m[:], xfp[:], axis=AX)
                oh = cp.tile([1, 4 * P], f32)
                nc.vector.memset(oh[:], 0.0)
                nc.vector.tensor_scalar(oh[0:1, 0:K], xfp[:], m[:], None,
                                        op0=OP.is_equal)
                ohT_ps = mps.tile([P, 4], f32, tag="mp")
                for i in range(4):
                    nc.tensor.transpose(ohT_ps[:, i:i + 1],
                                        oh[0:1, i * P:(i + 1) * P], ones11[:])
                ohT = cp.tile([P, 4], bf16)
                nc.vector.tensor_copy(ohT[:], ohT_ps[:])

                # w_selT[c] = proj_weight[top_cls, c], [128, CT] c-tile cols
                wsel_ps = mps.tile([P, CT], f32, tag="mp")
                for ct in range(CT):
                    for kc in range(4):
                        kcnt = min(P, K - kc * P)
                        nc.tensor.matmul(
                            wsel_ps[:, ct:ct + 1],
                            proj_sb[0:kcnt,
                                    kc * C + ct * P:kc * C + (ct + 1) * P],
                            ohT[0:kcnt, kc:kc + 1],
                            start=(kc == 0), stop=(kc == 3))
                # wsel8[p, two*16 + ctp] = 64 * wsel[p, 2*ctp+two], fp8
                wsel8 = cp.tile([P, 32], fp8)
                wv_out = wsel8[:].rearrange("p (two q) -> p two q", two=2)
                wv_in = wsel_ps[:].rearrange("p (q two) -> p two q", two=2)
                nc.vector.tensor_scalar(wv_out[:, :, 0:CTP], wv_in, 64.0,
                                        None, op0=OP.mult)

                # cam[1, 784] = w_sel @ x (center tap), fp8 DoubleRow
                cam_ps = [mps.tile([1, NF], f32, tag="mp", name=f"cam_ps{_h}")
                          for _h in range(NH)]
                for nh in range(NH):
                    for ctp in range(CTP):
                        nc.tensor.matmul(
                            cam_ps[nh][:],
                            wv_out[:, :, ctp:ctp + 1],
                            conv_rhs(ctp, 4, nh),
                            start=(ctp == 0), stop=(ctp == CTP - 1),
                            perf_mode=DR)
                cam_row = cp.tile([1, PADN], f32)
                for nh in range(NH):
                    nc.vector.tensor_copy(
                        cam_row[0:1, nh * NF:(nh + 1) * NF], cam_ps[nh][:])

                cmin = cp.tile([1, 1], f32)
                cmax = cp.tile([1, 1], f32)
                nc.vector.tensor_reduce(cmin[:], cam_row[0:1, 0:NPOS],
                                        axis=AX, op=OP.min)
                nc.vector.reduce_max(cmax[:], cam_row[0:1, 0:NPOS], axis=AX)
                rng_t = cp.tile([1, 1], f32)
                nc.vector.tensor_scalar(rng_t[:], cmax[:], cmin[:], None,
                                        op0=OP.subtract)
                inv = cp.tile([1, 1], f32)
                nc.vector.reciprocal(inv[:], rng_t[:])

                camn = cp.tile([1, PADN], f32)
                nc.vector.memset(camn[:], NEG)
                nc.vector.tensor_scalar(camn[0:1, 0:NPOS],
                                        cam_row[0:1, 0:NPOS],
                                        cmin[:], inv[:],
                                        op0=OP.subtract, op1=OP.mult)

                # broadcast camn across partitions: camB[128, 784]
                camB = cp.tile([P, NPOS], f32)
                for nh in range(NH):
                    cb_ps = mps.tile([P, NF], f32, tag="mp")
                    nc.tensor.matmul(cb_ps[:], ones_row[:],
                                     camn[0:1, nh * NF:(nh + 1) * NF],
                                     start=True, stop=True)
                    nc.vector.tensor_copy(
                        camB[:, nh * NF:(nh + 1) * NF], cb_ps[:])

                # camn in partition layout [128, 7]
                cnp_ps = mps.tile([P, 7], f32, tag="mp")
                for a in range(7):
                    nc.tensor.transpose(cnp_ps[:, a:a + 1],
                                        camn[0:1, a * P:(a + 1) * P],
                                        ones11[:])
                camnP = cp.tile([P, 7], f32)
                nc.vector.tensor_copy(camnP[:], cnp_ps[:])

                # rank[p,a] = #{j: camn[j] >= camn[p,a]}; top-392 = rank<=392
                ge = cp.tile([P, NPOS], f32)
                rank = cp.tile([P, 7], f32)
                for a in range(7):
                    nc.vector.tensor_scalar(ge[:], camB[:],
                                            camnP[:, a:a + 1],
                                            None, op0=OP.is_ge, op1=OP.add,
                                            accum_out=rank[:, a:a + 1])
                maskP = cp.tile([P, 7], f32)
                nc.vector.tensor_scalar(maskP[:], rank[:], float(N_TOKEN),
                                        None, op0=OP.is_le)
                yP = cp.tile([P, 7], f32)
                nc.vector.tensor_mul(yP[:], maskP[:], camnP[:])
                fe["yP"] = yP

            # ---------- conv main loop (fp8 DoubleRow) ----------
            s_ps = [sps.tile([1, NF], f32, tag=f"s{nh}", name=f"s_ps{nh}")
                    for nh in range(NH)]

            def emit_conv_dt(dt):
                ps = [cps.tile([P, NF], f32, tag=f"cv{nh}",
                               name=f"ps{dt}_{nh}")
                      for nh in range(NH)]
                if dt == 0:
                    wtile = None
                else:
                    wtile = wpb.tile([P, CTP * 9 * 2 * P], fp8, name="w_big",
                                     tag="w_big")
                    nc.sync.dma_start(wtile[:], w8_d[dt])
                for ctp in range(CTP):
                    if dt == 0:
                        w_ct = wp.tile([P, 9 * 2 * P], fp8, name="w_ct",
                                       tag="w_ct")
                        nc.sync.dma_start(
                            w_ct[:],
                            w8_d[dt][:, ctp * 9 * 2 * P:
                                     (ctp + 1) * 9 * 2 * P])
                        nc.sync.dma_start(xp8tiles[ctp][:], xp8_d[ctp])
                    for tap in range(9):
                        if dt == 0:
                            wsl = w_ct[:, tap * 2 * P:(tap + 1) * 2 * P]
                        else:
                            wsl = wtile[:, (ctp * 9 + tap) * 2 * P:
                                        (ctp * 9 + tap + 1) * 2 * P]
                        lhsT3 = wsl.rearrange("p (two q) -> p two q", two=2)
                        for nh in range(NH):
                            nc.tensor.matmul(
                                ps[nh][:], lhsT3, conv_rhs(ctp, tap, nh),
                                start=(ctp == 0 and tap == 0),
                                stop=(ctp == CTP - 1 and tap == 8),
                                perf_mode=DR)
                for nh in range(NH):
                    relu_t = rp.tile([P, NF], bf16, name="relu_t")
                    nc.scalar.activation(relu_t[:], ps[nh][:], AF.Relu,
                                         bias=cb_sb[:, dt:dt + 1],
                                         scale=1.0 / 64.0)
                    nc.tensor.matmul(s_ps[nh][:], sw_sb[:, dt:dt + 1],
                                     relu_t[:],
                                     start=(dt == 0), stop=(dt == DT - 1))

            emit_conv_dt(0)
            emit_conv_dt(1)
            emit_frontend()
            yP = fe["yP"]
            emit_conv_dt(2)
            emit_conv_dt(3)

            # ---------- epilogue: BCE = sum softplus(xcam) - sum xcam*y ----
            xcam_row = cp.tile([1, PADN], f32)
            nc.vector.memset(xcam_row[:], 0.0)
            for nh in range(NH):
                nc.vector.tensor_scalar(
                    xcam_row[0:1, nh * NF:(nh + 1) * NF], s_ps[nh][:],
                    sb_sb[:], None, op0=OP.add)

            et = cp.tile([1, NPOS], f32)
            nc.scalar.activation(et[:], xcam_row[0:1, 0:NPOS], AF.Exp)
            sp = cp.tile([1, NPOS], f32)
            sp_sum = cp.tile([1, 1], f32)
            nc.scalar.activation(sp[:], et[:], AF.Ln, bias=1.0,
                                 accum_out=sp_sum[:])

            xcp_ps = mps.tile([P, 7], f32, tag="mp")
            for a in range(7):
                nc.tensor.transpose(xcp_ps[:, a:a + 1],
                                    xcam_row[0:1, a * P:(a + 1) * P],
                                    ones11[:])
            xcamP = cp.tile([P, 7], f32)
            nc.vector.tensor_copy(xcamP[:], xcp_ps[:])

            prodP = cp.tile([P, 7], f32)
            nc.vector.tensor_mul(prodP[:], yP[:], xcamP[:])
            partial = cp.tile([P, 1], f32)
            nc.vector.reduce_sum(partial[:], prodP[:], axis=AX)

            dot_ps = mps.tile([1, 1], f32, tag="mp")
            nc.tensor.matmul(dot_ps[:], ones_col[:], partial[:],
                             start=True, stop=True)

            final = cp.tile([1, 1], f32)
            nc.vector.tensor_scalar(final[:], dot_ps[:], -1.0, sp_sum[:],
                                    op0=OP.mult, op1=OP.add)
            nc.sync.dma_start(out_d[:], final[:])

    nc.compile()
    return nc


def _prep_in_maps(x, x_fpv_pred, proj_weight, conv1_w, conv1_b, score_w,
                  score_b):
    import concourse.mybir as mybir
    bf16 = ml_dtypes.bfloat16
    fp8 = mybir.dt.np(mybir.dt.float8e4)

    # padded planes [B, CT, P, T, 9, 9] then 3 w-shifted 9x7 copies
    xr = np.asarray(x, np.float32).reshape(B, CT, P, T, H, W)
    xp9 = np.zeros((B, CT, P, T, 9, 9), np.float32)
    xp9[:, :, :, :, 1:8, 1:8] = xr
    xp9 = xp9.reshape(B, CTP, 2, P, T, 9, 9)
    # x3[b, ctp, two, p, s, t, h', w] = xp9[b, ctp, two, p, t, h', w+s]
    x3 = np.stack([xp9[..., s:s + 7] for s in range(3)], axis=4)
    # dims now (b, ctp, two, p, s, t, h', w) -> (b, ctp, p, two, s, t, h', w)
    xp8 = np.ascontiguousarray(
        x3.transpose(0, 1, 3, 2, 4, 5, 6, 7).reshape(B, CTP, P, XF)
    ).astype(fp8)

    w9 = np.asarray(conv1_w, np.float32).reshape(D, C, 9)
    # w8[dt, p, ((ctp*9 + tap)*2 + two)*P + q]
    #   = 64 * conv1_w[dt*P+q, (2*ctp+two)*P+p, tap]
    w8 = np.ascontiguousarray(
        (w9 * 64.0).reshape(DT, P, CTP, 2, P, 9).transpose(0, 4, 2, 5, 3, 1)
        .reshape(DT, P, CTP * 9 * 2 * P)).astype(fp8)

    proj_bf = np.asarray(proj_weight, np.float32).astype(bf16)
    cb = np.ascontiguousarray(
        np.asarray(conv1_b, np.float32).reshape(DT, P).T)
    sw = np.ascontiguousarray(
        np.asarray(score_w, np.float32).reshape(DT, P).T).astype(bf16)
    sb = np.asarray(score_b, np.float32).reshape(1, 1)
    xfp = np.asarray(x_fpv_pred, np.float32)

    in_maps = []
    for b in range(B):
        in_maps.append({
            "xp8": xp8[b],
            "w8": w8,
            "proj": proj_bf,
            "xfp": np.ascontiguousarray(xfp[b:b + 1]),
            "cb": cb,
            "sw": sw,
            "sb": sb,
        })
    return in_maps


def run(inputs, trace=False):
    """Build (cached), run on 8 cores, return (loss, BassKernelResults)."""
    from concourse.bass_utils import run_bass_kernel_spmd

    if "nc" not in _cache:
        _cache["nc"] = _build_nc()
    nc = _cache["nc"]
    in_maps = _prep_in_maps(**inputs)
    res = run_bass_kernel_spmd(nc, in_maps, core_ids=list(range(B)),
                               trace=trace)
    total = sum(float(np.asarray(res.results[b]["out"])[0, 0])
                for b in range(B))
    loss = np.float32(total / float(B * T * H * W))
    return loss, res


def kernel(**inputs):
    loss, _ = run(inputs, trace=False)
    return loss
